# revision 7
# baseline (speedup 1.0000x reference)
"""ColBERT maxsim scoring kernel for Trainium2 (8 NeuronCores, SPMD).

Problem: Q [128, 32, 128] f32, D [1024, 220, 128] f32, D_mask [1024, 220] i32,
nway=8.  out[b] = sum_q max_k where(mask[b,k], D[b] @ Q[b//8].T, -9999)[k, q]
for b in 0..1024.

Sharding: data-parallel over docs. Core c handles docs [128c, 128c+128) and
the matching 16 query batches.

Host-side prep (per core):
  - Padded doc-token rows of D are replaced by a copy of the doc's first
    real token row: duplicates never change the per-doc max, so no mask
    bias is needed on-device at all.  (Fully-padded docs -- impossible for
    this input distribution -- are patched on the host afterwards.)
  - D is cast to bf16 and pre-transposed to [DIM, 28160 doc-rows] so the
    device DMA is a plain contiguous stream; no on-chip transposes.
  - Q is pre-transposed to [DIM, 512] (16 groups x 32 queries).

Per-core device program:
  - 16 chunk DMAs (one query group = 8 docs = 1760 columns of D^T each,
    alternating sync/scalar HWDGE queues) stream D^T into SBUF.  Fine
    chunking lets the PE start after ~1/8 of the DMA instead of waiting
    for most of it.
  - Per group g: 4 col-tiled matmuls (tile_position=(0,32j), each its own
    start/stop accumulation group) put scores for doc pair (8g+2j, 8g+2j+1)
    in psum bank partitions [32j, 32j+32) -> [128, 440] bank.
  - One 3D reduce_max per group ([128, 2, 220] -> [128, 2]) -> Mx [128, 32].
  - Mx is DMA'd out directly; the host sums the four 32-query partition
    blocks and de-interleaves docs.
"""

import numpy as np
import ml_dtypes

import concourse.bacc as bacc
import concourse.mybir as mybir
from concourse import bass_utils
from concourse.tile import TileContext

F32 = mybir.dt.float32
BF16 = mybir.dt.bfloat16
FP8 = mybir.dt.float8e3

N_CORES = 8
B = 128          # query batches
QLEN = 32
DIM = 128
NWAY = 8
DLEN = 220
DOCS_PER_CORE = (B * NWAY) // N_CORES          # 128
ROWS_PER_CORE = DOCS_PER_CORE * DLEN           # 28160
GROUPS_PER_CORE = DOCS_PER_CORE // NWAY        # 16
GROW = NWAY * DLEN                             # 1760 rows per group
GCOLS = 2 * DLEN                               # 440 score cols per psum bank
BIG = 9999.0

_CACHE = {}


def _build_module():
    """Trace + compile the per-core bass module (same program on all cores)."""
    if "nc" in _CACHE:
        return _CACHE["nc"]

    nc = bacc.Bacc("TRN2", target_bir_lowering=False, debug=False)

    dt_dram = nc.dram_tensor("dt_in", [DIM, ROWS_PER_CORE], FP8,
                             kind="ExternalInput")
    qt_dram = nc.dram_tensor("qt_in", [DIM, GROUPS_PER_CORE * QLEN], BF16,
                             kind="ExternalInput")
    out_dram = nc.dram_tensor("outp", [128, 32], F32, kind="ExternalOutput")

    N_CHUNK = 4
    GPC = GROUPS_PER_CORE // N_CHUNK               # groups per chunk
    with TileContext(nc) as tc:
        with (
            tc.tile_pool(name="const", bufs=1) as cpool,
            tc.tile_pool(name="dt", bufs=N_CHUNK) as dt_pool,
            tc.tile_pool(name="score", bufs=8, space="PSUM") as score_pool,
        ):
            qt = cpool.tile([128, GROUPS_PER_CORE * QLEN], BF16)
            nc.scalar.dma_start(out=qt[:, :], in_=qt_dram.ap())

            mx = cpool.tile([128, 32], F32)

            # all chunk DMAs issued up-front, back-to-back per queue
            dts = []
            for ch in range(N_CHUNK):
                dtile = dt_pool.tile([128, GPC * GROW], FP8)
                eng = nc.sync if ch % 2 == 0 else nc.scalar
                eng.dma_start(
                    out=dtile[:, :],
                    in_=dt_dram.ap()[:, ch * GPC * GROW:(ch + 1) * GPC * GROW],
                )
                dts.append(dtile)

            for g in range(GROUPS_PER_CORE):
                dtile = dts[g // GPC]
                c0 = (g % GPC) * GROW
                ps = score_pool.tile([128, GCOLS], F32)
                for j in range(4):
                    nc.tensor.matmul(
                        ps[32 * j:32 * (j + 1), :],
                        lhsT=qt[:, QLEN * g:QLEN * (g + 1)],
                        rhs=dtile[:, c0 + GCOLS * j:c0 + GCOLS * (j + 1)],
                        start=True, stop=True,
                        tile_position=(0, 32 * j),
                        skip_group_check=True,
                    )
                nc.vector.tensor_reduce(
                    mx[:, 2 * g:2 * g + 2],
                    ps[:, :].rearrange("p (t k) -> p t k", t=2),
                    axis=mybir.AxisListType.X,
                    op=mybir.AluOpType.max,
                )
            nc.sync.dma_start(out=out_dram.ap(), in_=mx[:, :])

    nc.compile()
    _CACHE["nc"] = nc
    return nc


def _in_maps(Q, D, D_mask):
    """Host-side prep: per-core input dicts (pad-fill + cast + transpose)."""
    mask = D_mask > 0
    first_real = np.argmax(mask, axis=1)                  # [1024]
    kk = np.arange(DLEN)[None, :]
    idx = np.where(mask, kk, first_real[:, None])         # [1024, 220]
    d_filled = np.take_along_axis(D, idx[:, :, None], axis=1)
    dt_all = np.ascontiguousarray(
        d_filled.reshape(N_CORES, ROWS_PER_CORE, DIM).transpose(0, 2, 1)
    ).astype(ml_dtypes.float8_e3m4)
    qt_all = np.ascontiguousarray(
        Q.reshape(N_CORES, GROUPS_PER_CORE * QLEN, DIM).transpose(0, 2, 1)
    ).astype(ml_dtypes.bfloat16)
    return [{"dt_in": dt_all[c], "qt_in": qt_all[c]} for c in range(N_CORES)]


def kernel(Q, D, D_mask, nway):
    assert int(nway) == NWAY
    Q = np.ascontiguousarray(np.asarray(Q, dtype=np.float32))
    D = np.ascontiguousarray(np.asarray(D, dtype=np.float32))
    D_mask = np.asarray(D_mask, dtype=np.int32)

    nc = _build_module()
    res = bass_utils.run_bass_kernel_spmd(nc, _in_maps(Q, D, D_mask),
                                          core_ids=list(range(N_CORES)))

    # outp[32j+q, 2g+t] = maxsim for doc (8g+2j+t), query q; sum over q.
    s = np.arange(32)
    j = np.arange(4)
    doc_idx = 8 * (s[None, :] // 2) + 2 * j[:, None] + (s[None, :] % 2)
    out = np.empty(B * NWAY, np.float32)
    for c in range(N_CORES):
        blk = res.results[c]["outp"].reshape(4, 32, 32).sum(axis=1)  # [j, s]
        per_core = np.empty(DOCS_PER_CORE, np.float32)
        per_core[doc_idx.ravel()] = blk.ravel()
        out[c * DOCS_PER_CORE:(c + 1) * DOCS_PER_CORE] = per_core

    # fully-padded docs: reference yields exactly 32 * -9999
    fully = ~(D_mask > 0).any(axis=1)
    if fully.any():
        out[fully] = np.float32(32 * -BIG)
    return out


# revision 8
# speedup vs baseline: 1.0176x; 1.0176x over previous
"""ColBERT maxsim scoring kernel for Trainium2 (8 NeuronCores, SPMD).

Problem: Q [128, 32, 128] f32, D [1024, 220, 128] f32, D_mask [1024, 220] i32,
nway=8.  out[b] = sum_q max_k where(mask[b,k], D[b] @ Q[b//8].T, -9999)[k, q]
for b in 0..1024.

Sharding: data-parallel over docs. Core c handles docs [128c, 128c+128) and
the matching 16 query batches.

Host-side prep (per core):
  - Padded doc-token rows of D are replaced by a copy of the doc's first
    real token row: duplicates never change the per-doc max, so no mask
    bias is needed on-device at all.  (Fully-padded docs -- impossible for
    this input distribution -- are patched on the host afterwards.)
  - D is cast to bf16 and pre-transposed to [DIM, 28160 doc-rows] so the
    device DMA is a plain contiguous stream; no on-chip transposes.
  - Q is pre-transposed to [DIM, 512] (16 groups x 32 queries).

Per-core device program:
  - 16 chunk DMAs (one query group = 8 docs = 1760 columns of D^T each,
    alternating sync/scalar HWDGE queues) stream D^T into SBUF.  Fine
    chunking lets the PE start after ~1/8 of the DMA instead of waiting
    for most of it.
  - Per group g: 4 col-tiled matmuls (tile_position=(0,32j), each its own
    start/stop accumulation group) put scores for doc pair (8g+2j, 8g+2j+1)
    in psum bank partitions [32j, 32j+32) -> [128, 440] bank.
  - One 3D reduce_max per group ([128, 2, 220] -> [128, 2]) -> Mx [128, 32].
  - Mx is DMA'd out directly; the host sums the four 32-query partition
    blocks and de-interleaves docs.
"""

import numpy as np
import ml_dtypes

import concourse.bacc as bacc
import concourse.mybir as mybir
from concourse import bass_utils
from concourse.tile import TileContext

F32 = mybir.dt.float32
BF16 = mybir.dt.bfloat16
FP8 = mybir.dt.float8e3

N_CORES = 8
B = 128          # query batches
QLEN = 32
DIM = 128
NWAY = 8
DLEN = 220
DOCS_PER_CORE = (B * NWAY) // N_CORES          # 128
ROWS_PER_CORE = DOCS_PER_CORE * DLEN           # 28160
GROUPS_PER_CORE = DOCS_PER_CORE // NWAY        # 16
GROW = NWAY * DLEN                             # 1760 rows per group
GCOLS = 2 * DLEN                               # 440 score cols per psum bank
BIG = 9999.0

_CACHE = {}


def _build_module():
    """Trace + compile the per-core bass module (same program on all cores)."""
    if "nc" in _CACHE:
        return _CACHE["nc"]

    nc = bacc.Bacc("TRN2", target_bir_lowering=False, debug=False)

    dt_dram = nc.dram_tensor("dt_in", [DIM, ROWS_PER_CORE], FP8,
                             kind="ExternalInput")
    qt_dram = nc.dram_tensor("qt_in", [DIM, GROUPS_PER_CORE * QLEN], BF16,
                             kind="ExternalInput")
    out_dram = nc.dram_tensor("outp", [128, 32], F32, kind="ExternalOutput")

    N_CHUNK = 8
    GPC = GROUPS_PER_CORE // N_CHUNK               # groups per chunk
    with TileContext(nc) as tc:
        with (
            tc.tile_pool(name="const", bufs=1) as cpool,
            tc.tile_pool(name="dt", bufs=N_CHUNK) as dt_pool,
            tc.tile_pool(name="score", bufs=4, space="PSUM") as score_pool,
        ):
            qt = cpool.tile([128, GROUPS_PER_CORE * QLEN], BF16)
            nc.scalar.dma_start(out=qt[:, :], in_=qt_dram.ap())

            mx = cpool.tile([128, 32], F32)

            # all chunk DMAs issued up-front, back-to-back per queue
            dts = []
            for ch in range(N_CHUNK):
                dtile = dt_pool.tile([128, GPC * GROW], FP8)
                eng = nc.sync if ch % 2 == 0 else nc.scalar
                eng.dma_start(
                    out=dtile[:, :],
                    in_=dt_dram.ap()[:, ch * GPC * GROW:(ch + 1) * GPC * GROW],
                )
                dts.append(dtile)

            for g in range(GROUPS_PER_CORE):
                dtile = dts[g // GPC]
                c0 = (g % GPC) * GROW
                ps = score_pool.tile([128, GCOLS], F32)
                for j in range(4):
                    nc.tensor.matmul(
                        ps[32 * j:32 * (j + 1), :],
                        lhsT=qt[:, QLEN * g:QLEN * (g + 1)],
                        rhs=dtile[:, c0 + GCOLS * j:c0 + GCOLS * (j + 1)],
                        start=True, stop=True,
                        tile_position=(0, 32 * j),
                        skip_group_check=True,
                    )
                nc.vector.tensor_reduce(
                    mx[:, 2 * g:2 * g + 2],
                    ps[:, :].rearrange("p (t k) -> p t k", t=2),
                    axis=mybir.AxisListType.X,
                    op=mybir.AluOpType.max,
                )
                if g == GROUPS_PER_CORE // 2 - 1:
                    nc.sync.dma_start(out=out_dram.ap()[:, 0:16],
                                      in_=mx[:, 0:16])
            nc.sync.dma_start(out=out_dram.ap()[:, 16:32], in_=mx[:, 16:32])

    nc.compile()
    _CACHE["nc"] = nc
    return nc


def _in_maps(Q, D, D_mask):
    """Host-side prep: per-core input dicts (pad-fill + cast + transpose)."""
    mask = D_mask > 0
    first_real = np.argmax(mask, axis=1)                  # [1024]
    kk = np.arange(DLEN)[None, :]
    idx = np.where(mask, kk, first_real[:, None])         # [1024, 220]
    d_filled = np.take_along_axis(D, idx[:, :, None], axis=1)
    dt_all = np.ascontiguousarray(
        d_filled.reshape(N_CORES, ROWS_PER_CORE, DIM).transpose(0, 2, 1)
    ).astype(ml_dtypes.float8_e3m4)
    qt_all = np.ascontiguousarray(
        Q.reshape(N_CORES, GROUPS_PER_CORE * QLEN, DIM).transpose(0, 2, 1)
    ).astype(ml_dtypes.bfloat16)
    return [{"dt_in": dt_all[c], "qt_in": qt_all[c]} for c in range(N_CORES)]


def kernel(Q, D, D_mask, nway):
    assert int(nway) == NWAY
    Q = np.ascontiguousarray(np.asarray(Q, dtype=np.float32))
    D = np.ascontiguousarray(np.asarray(D, dtype=np.float32))
    D_mask = np.asarray(D_mask, dtype=np.int32)

    nc = _build_module()
    res = bass_utils.run_bass_kernel_spmd(nc, _in_maps(Q, D, D_mask),
                                          core_ids=list(range(N_CORES)))

    # outp[32j+q, 2g+t] = maxsim for doc (8g+2j+t), query q; sum over q.
    s = np.arange(32)
    j = np.arange(4)
    doc_idx = 8 * (s[None, :] // 2) + 2 * j[:, None] + (s[None, :] % 2)
    out = np.empty(B * NWAY, np.float32)
    for c in range(N_CORES):
        blk = res.results[c]["outp"].reshape(4, 32, 32).sum(axis=1)  # [j, s]
        per_core = np.empty(DOCS_PER_CORE, np.float32)
        per_core[doc_idx.ravel()] = blk.ravel()
        out[c * DOCS_PER_CORE:(c + 1) * DOCS_PER_CORE] = per_core

    # fully-padded docs: reference yields exactly 32 * -9999
    fully = ~(D_mask > 0).any(axis=1)
    if fully.any():
        out[fully] = np.float32(32 * -BIG)
    return out


# revision 11
# speedup vs baseline: 1.0571x; 1.0388x over previous
"""ColBERT maxsim scoring kernel for Trainium2 (8 NeuronCores, SPMD).

Problem: Q [128, 32, 128] f32, D [1024, 220, 128] f32, D_mask [1024, 220] i32,
nway=8.  out[b] = sum_q max_k where(mask[b,k], D[b] @ Q[b//8].T, -9999)[k, q]
for b in 0..1024.

Sharding: data-parallel over docs. Core c handles docs [128c, 128c+128) and
the matching 16 query batches.

Host-side prep (per core):
  - Padded doc-token rows of D are replaced by a copy of the doc's first
    real token row: duplicates never change the per-doc max, so no mask
    bias is needed on-device at all.  (Fully-padded docs -- impossible for
    this input distribution -- are patched on the host afterwards.)
  - D is cast to fp8 e3m4 (randn fits comfortably in its +-15.5 range;
    measured end-to-end rel err 2.9e-3 vs the 2e-2 gate) and pre-transposed
    to [DIM, 28160 doc-rows] so the device DMA is a plain contiguous
    stream; no on-chip transposes.  fp8 halves HBM traffic vs bf16.
  - Q is pre-transposed to [DIM, 512] bf16 (16 groups x 32 queries).

Per-core device program (raw bass engine streams, manual semaphores -- the
Tile framework's auto-semaphore teardown alone cost ~7us):
  - 8 chunk DMAs (2 query groups each, alternating sync/scalar HWDGE
    queues) stream D^T into SBUF, issued as each queue's first
    instructions.
  - Tensor stream: per group g, 4 col-tiled matmuls (tile_position=
    (0,32j)) put scores for doc pair (8g+2j, 8g+2j+1) in psum bank (g%8)
    partitions [32j, 32j+32); the last matmul bumps s_mm.
  - Vector stream: per group, one 3D reduce_max ([128, 2, 220] ->
    [128, 2]) into Mx [128, 32]; bumps s_red (also recycles the bank).
  - Sync stream DMAs Mx halves out after groups 7 and 15; the host sums
    the four 32-query partition blocks and de-interleaves docs.
"""

import numpy as np
import ml_dtypes

import concourse.bacc as bacc
import concourse.mybir as mybir
from concourse import bass_utils

F32 = mybir.dt.float32
BF16 = mybir.dt.bfloat16
FP8 = mybir.dt.float8e3

N_CORES = 8
B = 128          # query batches
QLEN = 32
DIM = 128
NWAY = 8
DLEN = 220
DOCS_PER_CORE = (B * NWAY) // N_CORES          # 128
ROWS_PER_CORE = DOCS_PER_CORE * DLEN           # 28160
GROUPS_PER_CORE = DOCS_PER_CORE // NWAY        # 16
GROW = NWAY * DLEN                             # 1760 rows per group
GCOLS = 2 * DLEN                               # 440 score cols per psum bank
N_CHUNK = 8
GPC = GROUPS_PER_CORE // N_CHUNK               # 2 groups per chunk
CHW = GPC * GROW                               # 3520 cols per chunk
BANK = 512                                     # psum bank stride (f32 elems)
BIG = 9999.0

_CACHE = {}


def _build_module():
    """Trace + compile the per-core bass module (same program on all cores)."""
    if "nc" in _CACHE:
        return _CACHE["nc"]

    nc = bacc.Bacc("TRN2", target_bir_lowering=False, debug=False)

    dt_dram = nc.dram_tensor("dt_in", [DIM, ROWS_PER_CORE], FP8,
                             kind="ExternalInput")
    qt_dram = nc.dram_tensor("qt_in", [DIM, GROUPS_PER_CORE * QLEN], BF16,
                             kind="ExternalInput")
    out_dram = nc.dram_tensor("outp", [128, 32], F32, kind="ExternalOutput")

    from contextlib import ExitStack
    with ExitStack() as stack, (
        nc.semaphore("q_dma")) as q_dma:
      ch_sems = [stack.enter_context(nc.semaphore(f"c_dma{i}"))
                 for i in range(N_CHUNK)]
      with (
        nc.semaphore("s_mm") as s_mm,      # groups of matmuls retired
        nc.semaphore("s_red") as s_red,    # reduces retired (bank recycle)
        nc.semaphore("s_out") as s_out,    # output DMA completions
        nc.sbuf_tensor("qt_sb", [128, GROUPS_PER_CORE * QLEN], BF16) as qt_sb,
        nc.sbuf_tensor("dt_sb", [128, ROWS_PER_CORE], FP8) as dt_sb,
        nc.sbuf_tensor("mx_sb", [128, 32], F32) as mx_sb,
        nc.psum_tensor("ps", [128, 8 * BANK], F32) as ps,
        nc.Block(no_gpsimd_drain=True) as block,
      ):
        @block.sync
        def _(sync):
            for ch in range(0, N_CHUNK, 2):
                sync.dma_start(
                    dt_sb[:, ch * CHW:(ch + 1) * CHW],
                    dt_dram.ap()[:, ch * CHW:(ch + 1) * CHW],
                ).then_inc(ch_sems[ch], 16)
            sync.wait_ge(s_red, GROUPS_PER_CORE // 2)
            sync.dma_start(out_dram.ap()[:, 0:16],
                           mx_sb[:, 0:16]).then_inc(s_out, 16)
            sync.wait_ge(s_red, GROUPS_PER_CORE)
            sync.dma_start(out_dram.ap()[:, 16:32],
                           mx_sb[:, 16:32]).then_inc(s_out, 16)
            sync.wait_ge(s_out, 32)

        @block.scalar
        def _(scalar):
            scalar.dma_start(qt_sb[:, :], qt_dram.ap()).then_inc(q_dma, 16)
            for ch in range(1, N_CHUNK, 2):
                scalar.dma_start(
                    dt_sb[:, ch * CHW:(ch + 1) * CHW],
                    dt_dram.ap()[:, ch * CHW:(ch + 1) * CHW],
                ).then_inc(ch_sems[ch], 16)

        @block.tensor
        def _(tensor):
            tensor.wait_ge(q_dma, 16)
            for g in range(GROUPS_PER_CORE):
                ch = g // GPC
                if g % GPC == 0:
                    tensor.wait_ge(ch_sems[ch], 16)
                if g >= 8:
                    tensor.wait_ge(s_red, g - 7)   # psum bank recycle
                b0 = (g % 8) * BANK
                c0 = g * GROW
                for j in range(4):
                    mm = tensor.matmul(
                        ps[32 * j:32 * (j + 1), b0:b0 + GCOLS],
                        lhsT=qt_sb[:, QLEN * g:QLEN * (g + 1)],
                        rhs=dt_sb[:, c0 + GCOLS * j:c0 + GCOLS * (j + 1)],
                        start=True, stop=True,
                        tile_position=(0, 32 * j),
                        skip_group_check=True,
                    )
                mm.then_inc(s_mm, 1)

        @block.vector
        def _(vector):
            for g in range(GROUPS_PER_CORE):
                vector.wait_ge(s_mm, g + 1)
                b0 = (g % 8) * BANK
                vector.tensor_reduce(
                    mx_sb[:, 2 * g:2 * g + 2],
                    ps[:, b0:b0 + GCOLS].rearrange("p (t k) -> p t k", t=2),
                    axis=mybir.AxisListType.X,
                    op=mybir.AluOpType.max,
                ).then_inc(s_red, 1)

    nc.compile()
    _CACHE["nc"] = nc
    return nc


def _in_maps(Q, D, D_mask):
    """Host-side prep: per-core input dicts (pad-fill + cast + transpose)."""
    mask = D_mask > 0
    first_real = np.argmax(mask, axis=1)                  # [1024]
    kk = np.arange(DLEN)[None, :]
    idx = np.where(mask, kk, first_real[:, None])         # [1024, 220]
    d_filled = np.take_along_axis(D, idx[:, :, None], axis=1)
    dt_all = np.ascontiguousarray(
        d_filled.reshape(N_CORES, ROWS_PER_CORE, DIM).transpose(0, 2, 1)
    ).astype(ml_dtypes.float8_e3m4)
    qt_all = np.ascontiguousarray(
        Q.reshape(N_CORES, GROUPS_PER_CORE * QLEN, DIM).transpose(0, 2, 1)
    ).astype(ml_dtypes.bfloat16)
    return [{"dt_in": dt_all[c], "qt_in": qt_all[c]} for c in range(N_CORES)]


def kernel(Q, D, D_mask, nway):
    assert int(nway) == NWAY
    Q = np.ascontiguousarray(np.asarray(Q, dtype=np.float32))
    D = np.ascontiguousarray(np.asarray(D, dtype=np.float32))
    D_mask = np.asarray(D_mask, dtype=np.int32)

    nc = _build_module()
    res = bass_utils.run_bass_kernel_spmd(nc, _in_maps(Q, D, D_mask),
                                          core_ids=list(range(N_CORES)))

    # outp[32j+q, 2g+t] = maxsim for doc (8g+2j+t), query q; sum over q.
    s = np.arange(32)
    j = np.arange(4)
    doc_idx = 8 * (s[None, :] // 2) + 2 * j[:, None] + (s[None, :] % 2)
    out = np.empty(B * NWAY, np.float32)
    for c in range(N_CORES):
        blk = res.results[c]["outp"].reshape(4, 32, 32).sum(axis=1)  # [j, s]
        per_core = np.empty(DOCS_PER_CORE, np.float32)
        per_core[doc_idx.ravel()] = blk.ravel()
        out[c * DOCS_PER_CORE:(c + 1) * DOCS_PER_CORE] = per_core

    # fully-padded docs: reference yields exactly 32 * -9999
    fully = ~(D_mask > 0).any(axis=1)
    if fully.any():
        out[fully] = np.float32(32 * -BIG)
    return out


# revision 13
# speedup vs baseline: 1.0739x; 1.0159x over previous
"""ColBERT maxsim scoring kernel for Trainium2 (8 NeuronCores, SPMD).

Problem: Q [128, 32, 128] f32, D [1024, 220, 128] f32, D_mask [1024, 220] i32,
nway=8.  out[b] = sum_q max_k where(mask[b,k], D[b] @ Q[b//8].T, -9999)[k, q]
for b in 0..1024.

Sharding: data-parallel over docs. Core c handles docs [128c, 128c+128) and
the matching 16 query batches.

Host-side prep (per core):
  - Padded doc-token rows of D are replaced by a copy of the doc's first
    real token row: duplicates never change the per-doc max, so no mask
    bias is needed on-device at all.  (Fully-padded docs -- impossible for
    this input distribution -- are patched on the host afterwards.)
  - D is cast to fp8 e3m4 (randn fits comfortably in its +-15.5 range;
    measured end-to-end rel err 2.9e-3 vs the 2e-2 gate) and pre-transposed
    to [DIM, 28160 doc-rows] so the device DMA is a plain contiguous
    stream; no on-chip transposes.  fp8 halves HBM traffic vs bf16.
  - Q is pre-transposed to [DIM, 512] bf16 (16 groups x 32 queries).

Per-core device program (raw bass engine streams, manual semaphores -- the
Tile framework's auto-semaphore teardown alone cost ~7us):
  - 8 chunk DMAs (2 query groups each, alternating sync/scalar HWDGE
    queues) stream D^T into SBUF, issued as each queue's first
    instructions.
  - Tensor stream: per group g, 4 col-tiled matmuls (tile_position=
    (0,32j)) put scores for doc pair (8g+2j, 8g+2j+1) in psum bank (g%8)
    partitions [32j, 32j+32); the last matmul bumps s_mm.
  - Vector stream: per group, one 3D reduce_max ([128, 2, 220] ->
    [128, 2]) into Mx [128, 32]; bumps s_red (also recycles the bank).
  - Sync stream DMAs Mx halves out after groups 7 and 15; the host sums
    the four 32-query partition blocks and de-interleaves docs.
"""

import numpy as np
import ml_dtypes

import concourse.bacc as bacc
import concourse.mybir as mybir
from concourse import bass_utils

F32 = mybir.dt.float32
BF16 = mybir.dt.bfloat16
FP8 = mybir.dt.float8e3

N_CORES = 8
B = 128          # query batches
QLEN = 32
DIM = 128
NWAY = 8
DLEN = 220
DOCS_PER_CORE = (B * NWAY) // N_CORES          # 128
ROWS_PER_CORE = DOCS_PER_CORE * DLEN           # 28160
GROUPS_PER_CORE = DOCS_PER_CORE // NWAY        # 16
GROW = NWAY * DLEN                             # 1760 rows per group
GCOLS = 2 * DLEN                               # 440 score cols per psum bank
N_CHUNK = 16
GPC = GROUPS_PER_CORE // N_CHUNK               # 1 group per chunk
CHW = GPC * GROW                               # 3520 cols per chunk
BANK = 512                                     # psum bank stride (f32 elems)
BIG = 9999.0

_CACHE = {}


def _build_module():
    """Trace + compile the per-core bass module (same program on all cores)."""
    if "nc" in _CACHE:
        return _CACHE["nc"]

    nc = bacc.Bacc("TRN2", target_bir_lowering=False, debug=False)

    dt_dram = nc.dram_tensor("dt_in", [DIM, ROWS_PER_CORE], FP8,
                             kind="ExternalInput")
    qt_dram = nc.dram_tensor("qt_in", [DIM, GROUPS_PER_CORE * QLEN], BF16,
                             kind="ExternalInput")
    out_dram = nc.dram_tensor("outp", [128, 32], F32, kind="ExternalOutput")

    from contextlib import ExitStack
    with ExitStack() as stack, (
        nc.semaphore("q_dma")) as q_dma:
      ch_sems = [stack.enter_context(nc.semaphore(f"c_dma{i}"))
                 for i in range(N_CHUNK)]
      with (
        nc.semaphore("s_mm") as s_mm,      # groups of matmuls retired
        nc.semaphore("s_red") as s_red,    # reduces retired (bank recycle)
        nc.semaphore("s_out") as s_out,    # output DMA completions
        nc.sbuf_tensor("qt_sb", [128, GROUPS_PER_CORE * QLEN], BF16) as qt_sb,
        nc.sbuf_tensor("dt_sb", [128, ROWS_PER_CORE], FP8) as dt_sb,
        nc.sbuf_tensor("mx_sb", [128, 32], F32) as mx_sb,
        nc.psum_tensor("ps", [128, 8 * BANK], F32) as ps,
        nc.Block(no_gpsimd_drain=True) as block,
      ):
        @block.sync
        def _(sync):
            # warmup: tiny transfer wakes the DMA engines early
            sync.dma_start(qt_sb[0:1, :], qt_dram.ap()[0:1, :]).then_inc(
                s_out, 16)
            for ch in range(0, N_CHUNK, 2):
                sync.dma_start(
                    dt_sb[:, ch * CHW:(ch + 1) * CHW],
                    dt_dram.ap()[:, ch * CHW:(ch + 1) * CHW],
                ).then_inc(ch_sems[ch], 16)
            for part in range(4):
                sync.wait_ge(s_red, 4 * (part + 1))
                sync.dma_start(out_dram.ap()[:, 8 * part:8 * (part + 1)],
                               mx_sb[:, 8 * part:8 * (part + 1)]
                               ).then_inc(s_out, 16)
            sync.wait_ge(s_out, 16 * 5)

        @block.scalar
        def _(scalar):
            scalar.dma_start(qt_sb[:, :], qt_dram.ap()).then_inc(q_dma, 16)
            for ch in range(1, N_CHUNK, 2):
                scalar.dma_start(
                    dt_sb[:, ch * CHW:(ch + 1) * CHW],
                    dt_dram.ap()[:, ch * CHW:(ch + 1) * CHW],
                ).then_inc(ch_sems[ch], 16)

        @block.tensor
        def _(tensor):
            tensor.wait_ge(q_dma, 16)
            for g in range(GROUPS_PER_CORE):
                ch = g // GPC
                if g % GPC == 0:
                    tensor.wait_ge(ch_sems[ch], 16)
                if g >= 8:
                    tensor.wait_ge(s_red, g - 7)   # psum bank recycle
                b0 = (g % 8) * BANK
                c0 = g * GROW
                for j in range(4):
                    mm = tensor.matmul(
                        ps[32 * j:32 * (j + 1), b0:b0 + GCOLS],
                        lhsT=qt_sb[:, QLEN * g:QLEN * (g + 1)],
                        rhs=dt_sb[:, c0 + GCOLS * j:c0 + GCOLS * (j + 1)],
                        start=True, stop=True,
                        tile_position=(0, 32 * j),
                        skip_group_check=True,
                    )
                mm.then_inc(s_mm, 1)

        @block.vector
        def _(vector):
            for g in range(GROUPS_PER_CORE):
                vector.wait_ge(s_mm, g + 1)
                b0 = (g % 8) * BANK
                vector.tensor_reduce(
                    mx_sb[:, 2 * g:2 * g + 2],
                    ps[:, b0:b0 + GCOLS].rearrange("p (t k) -> p t k", t=2),
                    axis=mybir.AxisListType.X,
                    op=mybir.AluOpType.max,
                ).then_inc(s_red, 1)

    nc.compile()
    _CACHE["nc"] = nc
    return nc


def _in_maps(Q, D, D_mask):
    """Host-side prep: per-core input dicts (pad-fill + cast + transpose)."""
    mask = D_mask > 0
    first_real = np.argmax(mask, axis=1)                  # [1024]
    kk = np.arange(DLEN)[None, :]
    idx = np.where(mask, kk, first_real[:, None])         # [1024, 220]
    d_filled = np.take_along_axis(D, idx[:, :, None], axis=1)
    dt_all = np.ascontiguousarray(
        d_filled.reshape(N_CORES, ROWS_PER_CORE, DIM).transpose(0, 2, 1)
    ).astype(ml_dtypes.float8_e3m4)
    qt_all = np.ascontiguousarray(
        Q.reshape(N_CORES, GROUPS_PER_CORE * QLEN, DIM).transpose(0, 2, 1)
    ).astype(ml_dtypes.bfloat16)
    return [{"dt_in": dt_all[c], "qt_in": qt_all[c]} for c in range(N_CORES)]


def kernel(Q, D, D_mask, nway):
    assert int(nway) == NWAY
    Q = np.ascontiguousarray(np.asarray(Q, dtype=np.float32))
    D = np.ascontiguousarray(np.asarray(D, dtype=np.float32))
    D_mask = np.asarray(D_mask, dtype=np.int32)

    nc = _build_module()
    res = bass_utils.run_bass_kernel_spmd(nc, _in_maps(Q, D, D_mask),
                                          core_ids=list(range(N_CORES)))

    # outp[32j+q, 2g+t] = maxsim for doc (8g+2j+t), query q; sum over q.
    s = np.arange(32)
    j = np.arange(4)
    doc_idx = 8 * (s[None, :] // 2) + 2 * j[:, None] + (s[None, :] % 2)
    out = np.empty(B * NWAY, np.float32)
    for c in range(N_CORES):
        blk = res.results[c]["outp"].reshape(4, 32, 32).sum(axis=1)  # [j, s]
        per_core = np.empty(DOCS_PER_CORE, np.float32)
        per_core[doc_idx.ravel()] = blk.ravel()
        out[c * DOCS_PER_CORE:(c + 1) * DOCS_PER_CORE] = per_core

    # fully-padded docs: reference yields exactly 32 * -9999
    fully = ~(D_mask > 0).any(axis=1)
    if fully.any():
        out[fully] = np.float32(32 * -BIG)
    return out
